# revision 6
# baseline (speedup 1.0000x reference)
"""CausalShapedAttention Trainium2 kernel (fp8 DoubleRow redesign).

y = beta * softmax(causal(q k^T / sqrt(hd))) @ v + alpha * v - gamma * MC @ v

where q,k = x @ W_attn^T (packed), v = x (reshaped to heads), MC = causal
uniform attention (row i: 1/(i+1) for j<=i).

Sharding: hybrid batch x head-quad: core c handles batch c//4 and heads
4*(c%4)..4*(c%4)+3.  Each core computes y[b, :, 256*(c%4) : 256*(c%4)+256].

Key techniques:
  - fp8e4m3 operands everywhere on the PE with DoubleRow perf mode
    (K=256 per instruction, 2x bf16 column rate) for the projection and
    for the P^T @ v accumulation (jb-block pairs).
  - scales: W is pre-scaled by 64 on the host (fp8 range), so
    scores_psum = 32768 * s_true; exp runs with scale=2^-15 folded in.
  - softmax denominator from a 65th ones-column (value 1/beta) in v16, so
    U[:, 64] = rowsum(exp)/beta and r1 = 1/U[:,64] gives beta/rowsum.
  - (MC @ v)_i = (sum_{j<=i} v_j)/(i+1): per-block tri matmul plus rank-1
    prefix (colsums accumulated on DVE), cinv = gamma/(i+1) host-side.
  - exp on ACT over 1024-wide (2-bank) psum tiles, fp8 output written
    straight into paired PT tiles [128, 2, W] ready to be DR stationaries.
  - fused tail on DVE/gpsimd: m1 = Lv*cinv - alpha*v ; y = U*r1 - m1.
"""

import os
import sys
import types

sys.path.insert(0, "/opt/trn_rl_repo")

import numpy as np
import ml_dtypes

B, T, C, H, HD = 2, 2048, 1024, 16, 64
NCORES = 8
HPC = 4                      # heads per core
TB = T // 128                # 16 row/col blocks

_PROGRAM = None
LAST_EXEC_NS = None
LAST_TRACE_DIR = None


def _install_patches():
    """Work around environment quirks:
    - walrus here rejects instructions with >1-2 sem waits (see
      _split_excess_waits).
    - antenv.axon_hooks is absent in this image: stub it and register the
      NTFF profile hook from trn_agent_boot so trace=True works.
    """
    try:
        import antenv  # noqa: F401
        if "antenv.axon_hooks" not in sys.modules:
            hooks_mod = types.ModuleType("antenv.axon_hooks")
            _h = [None]
            hooks_mod.set_axon_ntff_profile_hook = lambda h: _h.__setitem__(0, h)
            hooks_mod.get_axon_ntff_profile_hook = lambda: _h[0]
            sys.modules["antenv.axon_hooks"] = hooks_mod
            antenv.axon_hooks = hooks_mod
            from trn_agent_boot.trn_boot import _ntff_profile_via_ctypes
            hooks_mod.set_axon_ntff_profile_hook(
                _ntff_profile_via_ctypes("/opt/axon/libaxon_pjrt.so")
            )
        import concourse.bass_utils as bu
        bu.upload_artifacts = lambda d: d  # no artifact bucket here
    except Exception:
        pass


def _split_excess_waits(nc, limit=1):
    """walrus here rejects instructions with more than ~2 sem waits; split
    excess waits onto same-engine NoOps inserted just before the instruction
    (engine streams are per-engine program order, so semantics are identical).
    """
    import concourse.mybir as mybir

    n = 0
    for bb in nc.main_func.blocks:
        out = []
        for inst in bb.instructions:
            si = inst.sync_info
            if (
                si is not None
                and si.on_wait
                and len(si.on_wait) > limit
                and inst.engine != mybir.EngineType.Unassigned
            ):
                waits = list(si.on_wait)
                for w in waits[:-limit]:
                    n += 1
                    nop = mybir.InstNoOp(
                        name=f"{inst.name}-wsplit{n}",
                        engine=inst.engine,
                        ins=[], outs=[],
                        sync_info=mybir.SyncInfo(on_wait=[w], on_update=[]),
                    )
                    nc.register_instruction(nop)
                    out.append(nop)
                inst.sync_info = mybir.SyncInfo(
                    on_wait=waits[-limit:], on_update=list(si.on_update)
                )
            out.append(inst)
        bb.instructions = out


def _build_program():
    import concourse.bass as bass
    import concourse.mybir as mybir
    import concourse.tile as tile
    from concourse.bass import ts, ds

    f32 = mybir.dt.float32
    fp8 = mybir.dt.float8e4
    Exp = mybir.ActivationFunctionType.Exp
    DR = mybir.MatmulPerfMode.DoubleRow
    mult = mybir.AluOpType.mult
    sub = mybir.AluOpType.subtract

    nc = bass.Bass()
    # DRAM inputs (packed, per-core; see _prep_inputs for layouts)
    xT8 = nc.dram_tensor("xT8", [128, 8, T], fp8, kind="ExternalInput")
    w8 = nc.dram_tensor("w8", [128, 4, 4, 2, 128], fp8, kind="ExternalInput")
    v16 = nc.dram_tensor("v16", [HPC, 128, TB, 65], fp8, kind="ExternalInput")
    v32 = nc.dram_tensor("v32", [HPC, 128, TB * 64], f32, kind="ExternalInput")
    tri_d = nc.dram_tensor("tri", [128, 128], fp8, kind="ExternalInput")
    cinv_d = nc.dram_tensor("cinv", [128, TB], f32, kind="ExternalInput")
    y = nc.dram_tensor("y", [T, HPC * 64], f32, kind="ExternalOutput")

    ESC = float(2.0 ** -15)  # exp scale: undoes host-side W*64 packing

    with tile.TileContext(nc) as tc:
        with (
            tc.tile_pool(name="consts", bufs=1) as consts,
            tc.tile_pool(name="xtp", bufs=1) as xtp,
            tc.tile_pool(name="qk", bufs=1) as qkp,
            tc.tile_pool(name="vp", bufs=1) as vp,
            tc.tile_pool(name="pt", bufs=1) as ptp,
            tc.tile_pool(name="pfx", bufs=4) as pfxp,
            tc.tile_pool(name="small", bufs=4) as small,
            tc.tile_pool(name="tmp", bufs=4) as tmp,
            tc.tile_pool(name="yst", bufs=1) as ystp,
            tc.tile_pool(name="sc_ps", bufs=2, space="PSUM") as sc_ps,
            tc.tile_pool(name="ul_ps", bufs=4, space="PSUM") as ul_ps,
        ):
            w8_t = consts.tile([128, 4, 4, 2, 128], fp8, tag="w8")
            nc.sync.dma_start(w8_t[:], w8[:])
            tri_t = consts.tile([128, 128], fp8, tag="tri")
            nc.sync.dma_start(tri_t[:], tri_d[:])
            cinv_t = consts.tile([128, TB], f32, tag="cinv")
            nc.sync.dma_start(cinv_t[:], cinv_d[:])

            xp = []
            for g in range(4):
                t = xtp.tile([128, 2, T], fp8, tag=f"x{g}", name=f"x{g}")
                nc.sync.dma_start(t[:], xT8[:, ds(2 * g, 2)])
                xp.append(t)

            v16_t = []
            v32_t = []
            for h in range(HPC):
                t = vp.tile([128, TB, 65], fp8, tag=f"v16_{h}",
                            name=f"v16_{h}")
                nc.sync.dma_start(t[:], v16[h])
                v16_t.append(t)
                t2 = vp.tile([128, TB * 64], f32, tag=f"v32_{h}",
                             name=f"v32_{h}")
                nc.sync.dma_start(t2[:], v32[h])
                v32_t.append(t2)

            # ---------------- projection (fp8 DoubleRow, K=1024) ----------
            # out m: 0=k01, 1=q01, 2=k23, 3=q23; each [128, T] fp8
            qk_t = [qkp.tile([128, T], fp8, tag=f"qk{m}", name=f"qk{m}")
                    for m in range(4)]
            for m in range(4):
                for n in range(4):
                    ps = sc_ps.tile([128, 1024], f32, tag="sp",
                                    name=f"pj{m}{n}")
                    for g in range(4):
                        nc.tensor.matmul(
                            ps[:, ds(0, 512)], w8_t[:, m, g],
                            xp[g][:, :, ts(n, 512)],
                            start=(g == 0), stop=(g == 3), perf_mode=DR,
                        )
                    # psum -> sbuf fp8 cast copies, spread across engines
                    dst = qk_t[m][:, ts(n, 512)]
                    src = ps[:, ds(0, 512)]
                    if m % 2 == 0:
                        nc.scalar.copy(dst, src)
                    else:
                        nc.vector.tensor_copy(dst, src)

            # ---------------- per-head prefix sums (for MC @ v) -----------
            pfx8 = {}
            for h in range(HPC):
                css = []
                for g in range(4):
                    cp = ul_ps.tile([1, 260], f32, tag="ul", name=f"cs{h}{g}")
                    nc.tensor.matmul(
                        cp[:], tri_t[:, ds(127, 1)], v16_t[h][:, ds(4 * g, 4)],
                        start=True, stop=True,
                    )
                    cs_sb = pfxp.tile([1, 260], f32, tag="cs_sb",
                                      name=f"cssb{h}{g}")
                    nc.vector.tensor_copy(cs_sb[:], cp[:])
                    css.append(cs_sb)
                prev = None
                for ib in range(1, TB):
                    s = css[(ib - 1) // 4][0:1, ds(((ib - 1) % 4) * 65, 65)]
                    a = pfxp.tile([1, 65], f32, tag="acc", name=f"acc{h}{ib}")
                    if prev is None:
                        nc.vector.tensor_copy(a[:], s)
                    else:
                        nc.vector.tensor_add(a[:], prev[:], s)
                    prev = a
                    p8t = pfxp.tile([1, 65], fp8, tag=f"pfx{ib}",
                                    name=f"pfx{h}{ib}")
                    nc.vector.tensor_copy(p8t[:], a[:])
                    pfx8[h, ib] = p8t

            yst = [ystp.tile([128, HPC * 64], f32, tag=f"yst{ib}",
                              name=f"yst{ib}") for ib in range(TB)]

            # ---------------- attention per head --------------------------
            for h in range(HPC):
                kt = qk_t[2 * (h // 2)]
                qt = qk_t[2 * (h // 2) + 1]
                p0 = 64 * (h % 2)
                # PT pair tiles [128, 2, Wg], Wg = T - 256 g; reused h -> h+2
                ptt = [ptp.tile([128, 2, T - 256 * g], fp8,
                                tag=f"pt{h % 2}_{g}", name=f"pt{h}_{g}")
                       for g in range(8)]  # noqa

                for jb in range(TB):
                    g, m = jb // 2, jb % 2
                    Wg = T - 256 * g
                    # scores S^T[j in jb, i] for i >= 128*jb, 1024-wide psum
                    for w2 in range(jb // 8, 2):
                        dcol = max(0, 128 * jb - 1024 * w2)
                        nw = 1024 - dcol
                        sp = sc_ps.tile([128, 1024], f32, tag="sp",
                                        name=f"sc{h}{jb}{w2}")
                        segs = ([(dcol, 512 - dcol), (512, 512)]
                                if dcol < 512 else [(dcol, 1024 - dcol)])
                        for (c0, nseg) in segs:
                            nc.tensor.matmul(
                                sp[:, ds(c0, nseg)],
                                kt[ds(p0, 64), ts(jb, 128)],
                                qt[ds(p0, 64), ds(1024 * w2 + c0, nseg)],
                                start=True, stop=True,
                            )
                        off = 1024 * w2 + dcol - 256 * g
                        nc.scalar.activation(
                            ptt[g][:, m, ds(off, nw)], sp[:, ds(dcol, nw)],
                            Exp, scale=ESC,
                        )
                    # causal mask on the diagonal block (i in jb-block)
                    dslc = ptt[g][:, m, ds(128 * m, 128)]
                    nc.vector.tensor_mul(dslc, dslc, tri_t[:])

                    # U[jb] = sum_{g' pairs} PT-pair^T @ v-pair  (DoubleRow)
                    ult = ul_ps.tile([128, 130], f32, tag="ul",
                                     name=f"ul{h}{jb}")
                    up = ult[:, ds(0, 65)]
                    npair = (jb + 1) // 2
                    for gg in range(npair):
                        nc.tensor.matmul(
                            up,
                            ptt[gg][:, :, ds(128 * jb - 256 * gg, 128)],
                            v16_t[h][:, ds(2 * gg, 2)],
                            start=(gg == 0),
                            stop=(gg == npair - 1 and m == 1),
                            perf_mode=DR,
                        )
                    if m == 0:  # own diagonal block: single fp8 matmul
                        nc.tensor.matmul(
                            up, ptt[g][:, 0, ds(0, 128)], v16_t[h][:, jb],
                            start=(jb == 0), stop=True,
                        )

                    # Lv[jb] = tri^T @ v16[jb] + ones x prefix
                    lp = ult[:, ds(65, 65)]
                    nc.tensor.matmul(
                        lp, tri_t[:], v16_t[h][:, jb],
                        start=True, stop=(jb == 0),
                    )
                    if jb > 0:
                        nc.tensor.matmul(
                            lp, tri_t[ds(0, 1), :], pfx8[h, jb][:],
                            start=False, stop=True,
                        )

                    # tail: y = U*r1 - (Lv*cinv - alpha*v)
                    r1 = small.tile([128, 1], f32, tag="r1",
                                     name=f"r1_{h}_{jb}")
                    nc.vector.reciprocal(r1[:], ult[:, ds(64, 1)])
                    m1 = tmp.tile([128, 64], f32, tag="m1",
                                   name=f"m1_{h}_{jb}")
                    nc.vector.scalar_tensor_tensor(
                        m1[:], ult[:, ds(65, 64)], cinv_t[:, ds(jb, 1)],
                        v32_t[h][:, ds(64 * jb, 64)], mult, sub,
                    )
                    nc.vector.scalar_tensor_tensor(
                        yst[jb][:, ds(64 * h, 64)],
                        ult[:, ds(0, 64)], r1[:], m1[:], mult, sub,
                    )
                    if h == HPC - 1:
                        nc.sync.dma_start(y[ts(jb, 128), :], yst[jb][:])

    _split_excess_waits(nc)
    nc.finalize()
    return nc


def _prep_inputs(x, W_attn, alpha, beta, gamma):
    """Host-side sharding/layout prep. Returns per-core input maps."""
    fp8 = ml_dtypes.float8_e4m3fn
    x = np.asarray(x, dtype=np.float32)
    W_attn = np.asarray(W_attn, dtype=np.float32)
    alpha = float(alpha)
    beta = float(beta)
    gamma = float(gamma)

    tri = np.triu(np.ones((128, 128), dtype=np.float32)).astype(fp8)  # j<=i
    cinv = gamma / (np.arange(1, T + 1, dtype=np.float32)
                    .reshape(TB, 128).T.copy())  # [p, ib]
    inv_beta = np.float32(1.0 / beta) if beta != 0 else np.float32(np.inf)

    in_maps = []
    for core in range(NCORES):
        b = core // 4
        h0 = HPC * (core % 4)
        # xT8[p, c, t] = x[b, t, 128c+p]
        xT8 = np.ascontiguousarray(
            x[b].T.reshape(8, 128, T).transpose(1, 0, 2)).astype(fp8)
        # w8[p, m, g, i, o]: m in (k01, q01, k23, q23), W cols (2g+i)*128+p
        w8 = np.empty((128, 4, 4, 2, 128), dtype=np.float32)
        for m in range(4):
            hh = h0 + 2 * (m // 2)
            rows = (C if m % 2 == 0 else 0) + np.arange(hh * 64,
                                                        (hh + 2) * 64)
            wm = W_attn[rows, :] * 64.0          # [128 outcols, C]
            # [o, (2g+i)*128+p] -> [p, g, i, o]
            w8[:, m] = wm.T.reshape(4, 2, 128, 128).transpose(2, 0, 1, 3)
        w8 = np.ascontiguousarray(w8).astype(fp8)

        v16 = np.empty((HPC, 128, TB, 65), dtype=np.float32)
        v32 = np.empty((HPC, 128, TB, 64), dtype=np.float32)
        for h in range(HPC):
            hh = h0 + h
            vb = x[b][:, hh * 64:(hh + 1) * 64].reshape(TB, 128, 64)
            v16[h, :, :, :64] = vb.transpose(1, 0, 2)
            v16[h, :, :, 64] = inv_beta
            v32[h] = alpha * vb.transpose(1, 0, 2)
        v16 = np.ascontiguousarray(v16).astype(fp8)
        v32 = np.ascontiguousarray(v32.reshape(HPC, 128, TB * 64))

        in_maps.append({
            "xT8": xT8,
            "w8": w8,
            "v16": v16,
            "v32": v32,
            "tri": tri,
            "cinv": cinv.astype(np.float32),
        })
    return in_maps


def kernel(x, W_attn, alpha, beta, gamma):
    global _PROGRAM, LAST_EXEC_NS, LAST_TRACE_DIR
    _install_patches()
    from concourse.bass_utils import run_bass_kernel_spmd

    if _PROGRAM is None:
        _PROGRAM = _build_program()
    nc = _PROGRAM

    in_maps = _prep_inputs(x, W_attn, alpha, beta, gamma)

    trace = os.environ.get("KERNEL_TRACE", "0") == "1"
    kwargs = {}
    if trace:
        trace_dir = os.environ.get("KERNEL_TRACE_DIR") or None
        if trace_dir:
            os.makedirs(trace_dir, exist_ok=True)
            kwargs["tmpdir"] = trace_dir
    res = run_bass_kernel_spmd(
        nc, in_maps, core_ids=list(range(NCORES)), trace=trace, **kwargs
    )
    LAST_EXEC_NS = res.exec_time_ns
    if trace and "tmpdir" in kwargs:
        LAST_TRACE_DIR = kwargs["tmpdir"]

    out = np.empty((B, T, C), dtype=np.float32)
    for core in range(NCORES):
        b = core // 4
        c0 = 256 * (core % 4)
        out[b, :, c0:c0 + 256] = res.results[core]["y"]
    return out
